# revision 29
# baseline (speedup 1.0000x reference)
"""Trainium2 Bass kernel for Enformer-style relative-position attention.

Problem: b=2, n=1536, dim=1536, 8 heads, dk=64, dv=192, rel-pos features=192.

Sharding: (batch x head-pair), no collectives. 8 cores = 2 batches x 4
head-pairs. Each core computes q/k/v projections for its 2 heads only,
attention over all 1536 query rows, and a partial output projection
(contraction over its 384 feature columns of Wout). The host sums the 4
partial outputs per batch and adds b_out.

All matmuls run in bf16 (full PE rate). rel_k^T (128 x 3071) is computed
once on-device; per-tile pre-shift bands are column slices of it. The
shifted band is folded into the content-logit PSUM with an
identity-matmul accumulation (psum += I @ rbt), so exp reads PSUM
directly and no separate vector add pass exists. attnv lags one block
behind the logit pipeline so the PE never waits for exp/transpose.

relative_shift is realized as a DRAM round trip: the pre-shift band
S_pre (128 x 1663) is written row-major to a flat DRAM scratch, and read
back with row stride 1662 starting at offset 127, which lands
shifted[p, j] = S_pre[p, 127 - p + j] exactly.
"""

import math
import os

import ml_dtypes
import numpy as np

import concourse.bass as bass
import concourse.mybir as mybir
import concourse.tile as tile
from concourse import bacc
from concourse.bass_utils import run_bass_kernel_spmd
from concourse.masks import make_identity

BF16 = ml_dtypes.bfloat16
F32 = mybir.dt.float32
BF = mybir.dt.bfloat16
AF = mybir.ActivationFunctionType

P = 128
N = 1536          # sequence length
D = 1536          # model dim
H = 8             # heads
DK = 64           # key dim per head
DV = 192          # value dim per head
NT = N // P       # q-tiles per core = 12
KC = D // P       # contraction chunks over model dim = 12
WB = N + P - 1    # pre-shift band width = 1663
RBS = WB - 1      # readback row stride = 1662
FLAT = P * WB     # flat scratch elements per (tile, head) = 212864
NPOS = 2 * N - 1  # 3071 relative positions
NRF = 192         # rel-pos feature size
HPD = 2 * DV      # head-pair value width = 384
PRE = 6           # stg blocks pre-issued ahead of the consume loop
NBLK = 2 * NT     # (tile, head) blocks per core = 24


def _np_positions():
    """numpy replication of reference.get_positional_embed(1536, 192)."""
    from scipy.special import gammaln as sp_gammaln

    n, feature_size = N, NRF
    dist = np.arange(-n + 1, n)
    adist = np.abs(dist).astype(np.float64)[:, None]
    num_basis = feature_size // 6
    max_range = math.log(n) / math.log(2.0)
    half_life = 2.0 ** np.linspace(3.0, max_range, num_basis)
    feat_exp = np.exp(-math.log(2.0) / half_life[None, :] * adist)
    center_widths = 2.0 ** np.arange(1, num_basis + 1) - 1.0
    feat_cm = (center_widths[None, :] > adist).astype(np.float64)
    stddev = n / (2 * num_basis)
    start_mean = n / num_basis
    mean = np.linspace(start_mean, float(n), num_basis)[None, :]
    concentration = (mean / stddev) ** 2
    rate = mean / (stddev**2)
    with np.errstate(divide="ignore", invalid="ignore"):
        xl = np.where(
            adist > 0,
            (concentration - 1.0) * np.log(np.where(adist > 0, adist, 1.0)),
            0.0,
        )
        xl = np.where((adist == 0) & (concentration - 1.0 != 0), -np.inf, xl)
    log_unnorm = xl - rate * adist
    log_norm = sp_gammaln(concentration) - concentration * np.log(rate)
    prob = np.exp(log_unnorm - log_norm) + 1e-8
    feat_gamma = prob / np.max(prob, axis=-1, keepdims=True)
    emb = np.concatenate([feat_exp, feat_cm, feat_gamma], axis=-1)
    sign = np.sign(dist).astype(np.float64)[:, None]
    return np.concatenate([emb, sign * emb], axis=-1).astype(np.float32)


def _build_nc():
    nc = bacc.Bacc("TRN2", target_bir_lowering=False)

    xt = nc.dram_tensor("xt", [D, N], BF, kind="ExternalInput")
    wq = nc.dram_tensor("wq", [D, P], BF, kind="ExternalInput")
    wk = nc.dram_tensor("wk", [D, P], BF, kind="ExternalInput")
    wv = nc.dram_tensor("wv", [D, HPD], BF, kind="ExternalInput")
    wo = nc.dram_tensor("wo", [HPD, D], BF, kind="ExternalInput")
    wra = nc.dram_tensor("wra", [P, P], BF, kind="ExternalInput")
    wrb = nc.dram_tensor("wrb", [NRF - P, P], BF, kind="ExternalInput")
    pta = nc.dram_tensor("pta", [P, NPOS], BF, kind="ExternalInput")
    ptb = nc.dram_tensor("ptb", [NRF - P, NPOS], BF, kind="ExternalInput")
    rcb = nc.dram_tensor("rcb", [P, 1], F32, kind="ExternalInput")
    rpb = nc.dram_tensor("rpb", [P, 1], F32, kind="ExternalInput")

    out = nc.dram_tensor("out", [N, D], BF, kind="ExternalOutput")

    scale = DK ** -0.5

    with tile.TileContext(nc) as tc:
        with (
            tc.tile_pool(name="res", bufs=1) as res,
            tc.tile_pool(name="ps_wide", bufs=1, space="PSUM") as pwide,
            tc.tile_pool(name="ps_stg", bufs=1, space="PSUM") as pstgp,
            tc.tile_pool(name="ps_sm", bufs=2, space="PSUM") as psml,
            tc.tile_pool(name="dram", bufs=8, space="DRAM") as dsc,
            tc.tile_pool(name="stg", bufs=7) as stgp,
            tc.tile_pool(name="rb", bufs=8) as rbp,
            tc.tile_pool(name="es", bufs=3) as esp,
            tc.tile_pool(name="at", bufs=3) as atp,
            tc.tile_pool(name="av", bufs=2) as avp,
            tc.tile_pool(name="avT", bufs=2) as avTp,
            tc.tile_pool(name="ou", bufs=2) as oup,
            tc.tile_pool(name="sm", bufs=4) as smp,
        ):
            # long-lived SBUF residents
            kT_sb = res.tile([P, N], BF, tag="kT")            # 3 KB/part
            qcT_sb = res.tile([P, N], BF, tag="qcT")          # 3 KB/part
            qpT_sb = res.tile([P, N], BF, tag="qpT")          # 3 KB/part
            v_sb = res.tile([P, KC * HPD], BF, tag="v")       # 9 KB/part
            wo_sb = res.tile([P, 3 * D], BF, tag="wo")        # 9 KB/part
            relk_sb = res.tile([P, NPOS + 1], BF, tag="relk")  # 6 KB/part
            ident_sb = res.tile([P, P], BF, tag="ident")
            rcb_sb = res.tile([P, 1], F32, tag="rcb")
            rpb_sb = res.tile([P, 1], F32, tag="rpb")

            make_identity(nc, ident_sb[:])

            rbts = {}
            slots = {}
            stg_tiles = {}

            def emit_stg_round(i, r):
                # pre-shift rel logits S_pre (128, 1663) in two psum rounds
                t, hl = divmod(i, 2)
                ho = hl * DK
                lo = (N - 1) - t * P - (P - 1)
                if r == 0:
                    stg_tiles[i] = stgp.tile([P, WB + 1], BF, tag="stg", name=f"stg{i}")
                stg = stg_tiles[i]
                ps = pstgp.tile([P, 1024], F32, tag="pstg", name=f"pstg{i}_{r}")
                base = r * 1024
                for c in range(2):
                    w = min(512, WB - base - c * 512)
                    nc.tensor.matmul(
                        ps[:, c * 512 : c * 512 + w],
                        qpT_sb[ho : ho + DK, t * P : (t + 1) * P],
                        relk_sb[ho : ho + DK, lo + base + c * 512 : lo + base + c * 512 + w],
                        start=True, stop=True,
                    )
                wr = min(1024, WB - base)
                nc.vector.tensor_copy(stg[:, base : base + wr], ps[:, :wr])
                if r == 1:
                    slot = dsc.tile([FLAT], BF, tag="slot", name=f"slot{i}")
                    nc.gpsimd.dma_start(
                        out=slot[:].rearrange("(p w) -> p w", w=WB), in_=stg[:, :WB]
                    )
                    slots[i] = slot

            def emit_readback(i):
                rbt = rbp.tile([P, N], BF, tag="rb", name=f"rbt{i}")
                nc.sync.dma_start(
                    out=rbt[:],
                    in_=slots.pop(i)[P - 1 : P - 1 + P * RBS].rearrange(
                        "(p w) -> p w", w=RBS
                    )[:, :N],
                )
                rbts[i] = rbt

            with tc.tile_pool(name="proj", bufs=1) as projp:
                xT_sb = projp.tile([P, KC * N], BF, tag="xT")       # 36 KB/part
                wq_sb = projp.tile([P, KC * P], BF, tag="wq")
                wk_sb = projp.tile([P, KC * P], BF, tag="wk")
                wv_sb = projp.tile([P, KC * HPD], BF, tag="wv")
                pta_sb = projp.tile([P, NPOS], BF, tag="pta")
                ptb_sb = projp.tile([NRF - P, NPOS], BF, tag="ptb")
                wra_sb = projp.tile([P, P], BF, tag="wra")
                wrb_sb = projp.tile([NRF - P, P], BF, tag="wrb")

                # scalar queue: q/k weights first so projections start early
                nc.scalar.dma_start(out=rcb_sb[:], in_=rcb[:])
                nc.scalar.dma_start(out=rpb_sb[:], in_=rpb[:])
                nc.scalar.dma_start(
                    out=wq_sb[:].rearrange("p (kc c) -> p kc c", c=P),
                    in_=wq[:].rearrange("(kc p) c -> p kc c", p=P),
                )
                nc.scalar.dma_start(
                    out=wk_sb[:].rearrange("p (kc c) -> p kc c", c=P),
                    in_=wk[:].rearrange("(kc p) c -> p kc c", p=P),
                )
                nc.scalar.dma_start(out=wra_sb[:], in_=wra[:])
                nc.scalar.dma_start(out=wrb_sb[:], in_=wrb[:])
                nc.scalar.dma_start(out=pta_sb[:], in_=pta[:])
                nc.scalar.dma_start(out=ptb_sb[:], in_=ptb[:])
                # sync queue: x chunks, then late weights
                for kc in range(KC):
                    nc.sync.dma_start(
                        out=xT_sb[:, kc * N : (kc + 1) * N],
                        in_=xt[kc * P : (kc + 1) * P, :],
                    )
                nc.sync.dma_start(
                    out=wv_sb[:].rearrange("p (kc c) -> p kc c", c=HPD),
                    in_=wv[:].rearrange("(kc p) c -> p kc c", p=P),
                )
                nc.sync.dma_start(
                    out=wo_sb[:].rearrange("p (cc c) -> p cc c", c=D),
                    in_=wo[:].rearrange("(cc p) c -> p cc c", p=P),
                )

                # ---- q projection (2 heads, all rows): psum [128, 1536] ----
                pq = pwide.tile([P, 3 * 512], F32, tag="pwide", name="pq")
                for kc in range(KC):
                    for c3 in range(3):
                        nc.tensor.matmul(
                            pq[:, c3 * 512 : (c3 + 1) * 512],
                            wq_sb[:, kc * P : (kc + 1) * P],
                            xT_sb[:, kc * N + c3 * 512 : kc * N + (c3 + 1) * 512],
                            start=(kc == 0), stop=(kc == KC - 1),
                        )
                nc.scalar.activation(
                    qcT_sb[:], pq[:], AF.Identity, bias=rcb_sb[:, 0:1], scale=scale,
                )
                nc.scalar.activation(
                    qpT_sb[:], pq[:], AF.Identity, bias=rpb_sb[:, 0:1], scale=scale,
                )

                # ---- k projection ----
                for c3 in range(3):
                    pk = psml.tile([P, 512], F32, tag="psml", name=f"pk{c3}")
                    for kc in range(KC):
                        nc.tensor.matmul(
                            pk[:],
                            wk_sb[:, kc * P : (kc + 1) * P],
                            xT_sb[:, kc * N + c3 * 512 : kc * N + (c3 + 1) * 512],
                            start=(kc == 0), stop=(kc == KC - 1),
                        )
                    nc.vector.tensor_copy(kT_sb[:, c3 * 512 : (c3 + 1) * 512], pk[:])

                # ---- rel_k projection: relk = Wrel_slice @ positions^T ----
                for c in range(6):
                    co = c * 512
                    w = min(512, NPOS - co)
                    pr = psml.tile([P, 512], F32, tag="psml", name=f"pr{c}")
                    nc.tensor.matmul(
                        pr[:, :w], wra_sb[:], pta_sb[:, co : co + w],
                        start=True, stop=False,
                    )
                    nc.tensor.matmul(
                        pr[:, :w], wrb_sb[:], ptb_sb[:, co : co + w],
                        start=False, stop=True,
                    )
                    nc.scalar.copy(relk_sb[:, co : co + w], pr[:, :w])

                # ---- v projection woven with pre-issued stg rounds ----
                for m in range(KC):
                    pvm = psml.tile([P, 512], F32, tag="psml", name=f"pv{m}")
                    for kc in range(KC):
                        nc.tensor.matmul(
                            pvm[:, :HPD],
                            xT_sb[:, kc * N + m * P : kc * N + (m + 1) * P],
                            wv_sb[:, kc * HPD : (kc + 1) * HPD],
                            start=(kc == 0), stop=(kc == KC - 1),
                        )
                    nc.vector.tensor_copy(v_sb[:, m * HPD : (m + 1) * HPD], pvm[:, :HPD])
                    if m < 2 * PRE:
                        emit_stg_round(m // 2, m % 2)
                        if m % 2 == 1:
                            emit_readback(m // 2)

            # ------------- attention + output, pipelined blocks -------------
            # block i computes logits+exp for (t, hl) = divmod(i, 2); attnv
            # for block i-1; output projection for tile t' after its second
            # head's attnv completes.
            att_alls = {}
            recips = {}
            avs = {}

            for i in range(NBLK + 1):
                if i < NBLK:
                    t, hl = divmod(i, 2)
                    ho = hl * DK
                    if i + PRE < NBLK:
                        emit_stg_round(i + PRE, 0)
                    # content logits into wide psum, band added via identity
                    pc = pwide.tile([P, 3 * 512], F32, tag="pwide", name=f"pc{i}")
                    for c3 in range(3):
                        nc.tensor.matmul(
                            pc[:, c3 * 512 : (c3 + 1) * 512],
                            qcT_sb[ho : ho + DK, t * P : (t + 1) * P],
                            kT_sb[ho : ho + DK, c3 * 512 : (c3 + 1) * 512],
                            start=True, stop=False,
                        )
                    rbt = rbts.pop(i)
                    for c3 in range(3):
                        sl = slice(c3 * 512, (c3 + 1) * 512)
                        nc.tensor.matmul(
                            pc[:, sl], ident_sb[:], rbt[:, sl],
                            start=False, stop=True,
                        )
                    if i + PRE < NBLK:
                        emit_stg_round(i + PRE, 1)
                    es = esp.tile([P, N], BF, tag="es", name=f"es{i}")
                    sums = smp.tile([P, 1], F32, tag="sums", name=f"sums{i}")
                    nc.scalar.activation(es[:], pc[:], AF.Exp, accum_out=sums[:])
                    recip = smp.tile([P, 1], F32, tag="recip", name=f"recip{i}")
                    nc.vector.reciprocal(recip[:], sums[:])
                    recips[i] = recip
                    # transpose in two halves so attnv can start on the
                    # first 6 key-chunks early; readback for block i+PRE
                    # queues on sync behind them (it has PRE blocks slack)
                    att_all = atp.tile([P, KC * P], BF, tag="at", name=f"at{i}")
                    nc.sync.dma_start_transpose(
                        att_all[:, : 6 * P].rearrange("p (kc c) -> p kc c", c=P),
                        es[:, : 6 * P],
                    )
                    nc.sync.dma_start_transpose(
                        att_all[:, 6 * P :].rearrange("p (kc c) -> p kc c", c=P),
                        es[:, 6 * P :],
                    )
                    att_alls[i] = att_all
                    if i + PRE < NBLK:
                        emit_readback(i + PRE)

                if i >= 1:
                    j = i - 1
                    tj, hj = divmod(j, 2)
                    if hj == 0:
                        avs[tj] = avp.tile([P, HPD], BF, tag="attnv", name=f"av{tj}")
                    att_all = att_alls.pop(j)
                    pv = psml.tile([P, 512], F32, tag="psml", name=f"pav{j}")
                    for kc in range(KC):
                        nc.tensor.matmul(
                            pv[:, :DV],
                            att_all[:, kc * P : (kc + 1) * P],
                            v_sb[:, kc * HPD + hj * DV : kc * HPD + (hj + 1) * DV],
                            start=(kc == 0), stop=(kc == KC - 1),
                        )
                    nc.scalar.activation(
                        avs[tj][:, hj * DV : (hj + 1) * DV], pv[:, :DV], AF.Copy,
                        scale=recips.pop(j)[:],
                    )
                    if hj == 1:
                        # ---- partial output projection for tile tj ----
                        attnv_sb = avs.pop(tj)
                        avT = avTp.tile([P, 3 * P], BF, tag="avT", name=f"avT{tj}")
                        nc.sync.dma_start_transpose(
                            avT[:].rearrange("p (cc c) -> p cc c", c=P), attnv_sb[:]
                        )
                        ot = oup.tile([P, D], BF, tag="ot", name=f"ot{tj}")
                        for c3 in range(3):
                            po = psml.tile([P, 512], F32, tag="psml", name=f"po{tj}_{c3}")
                            for cc in range(3):
                                nc.tensor.matmul(
                                    po[:],
                                    avT[:, cc * P : (cc + 1) * P],
                                    wo_sb[:, cc * D + c3 * 512 : cc * D + (c3 + 1) * 512],
                                    start=(cc == 0), stop=(cc == 2),
                                )
                            nc.scalar.activation(
                                ot[:, c3 * 512 : (c3 + 1) * 512], po[:], AF.Copy,
                            )
                        nc.gpsimd.dma_start(
                            out=out[tj * P : (tj + 1) * P, :], in_=ot[:]
                        )

    nc.compile()
    return nc


_CACHE = {}


def _get_nc():
    if "nc" not in _CACHE:
        _CACHE["nc"] = _build_nc()
    return _CACHE["nc"]


def kernel(x, Wq, Wk, Wv, Wrel, Wout, b_out, rel_content_bias, rel_pos_bias):
    x = np.asarray(x, np.float32)
    Wq = np.asarray(Wq, np.float32)
    Wk = np.asarray(Wk, np.float32)
    Wv = np.asarray(Wv, np.float32)
    Wrel = np.asarray(Wrel, np.float32)
    Wout = np.asarray(Wout, np.float32)
    b_out = np.asarray(b_out, np.float32)
    rcb = np.asarray(rel_content_bias, np.float32).reshape(H, DK)
    rpb = np.asarray(rel_pos_bias, np.float32).reshape(H, DK)

    positions = _np_positions()  # (3071, 192) f32, input-independent constant
    posT = np.ascontiguousarray(positions.T).astype(BF16)  # (192, 3071)

    in_maps = []
    for core in range(8):
        bi, hp = core // 4, core % 4
        h0 = 2 * hp
        xT = np.ascontiguousarray(x[bi].T).astype(BF16)  # (dim, n)
        wrT = np.ascontiguousarray(
            Wrel[h0 * DK : (h0 + 2) * DK, :].T
        ).astype(BF16)  # (192, 128)
        in_maps.append(
            {
                "xt": xT,
                "wq": np.ascontiguousarray(Wq[h0 * DK : (h0 + 2) * DK, :].T).astype(BF16),
                "wk": np.ascontiguousarray(Wk[h0 * DK : (h0 + 2) * DK, :].T).astype(BF16),
                "wv": np.ascontiguousarray(Wv[h0 * DV : (h0 + 2) * DV, :].T).astype(BF16),
                "wo": np.ascontiguousarray(Wout[:, h0 * DV : (h0 + 2) * DV].T).astype(BF16),
                "wra": np.ascontiguousarray(wrT[:P]),
                "wrb": np.ascontiguousarray(wrT[P:]),
                "pta": np.ascontiguousarray(posT[:P]),
                "ptb": np.ascontiguousarray(posT[P:]),
                "rcb": np.ascontiguousarray(rcb[h0 : h0 + 2].reshape(P, 1)),
                "rpb": np.ascontiguousarray(rpb[h0 : h0 + 2].reshape(P, 1)),
            }
        )

    nc = _get_nc()
    trace = bool(os.environ.get("KERNEL_TRACE"))
    res = run_bass_kernel_spmd(nc, in_maps, list(range(8)), trace=trace)
    _CACHE["last_res"] = res

    out = np.zeros((2, N, D), np.float32)
    for core in range(8):
        bi = core // 4
        out[bi] += np.asarray(res.results[core]["out"]).astype(np.float32)
    out += b_out
    return out


# revision 32
# speedup vs baseline: 1.0525x; 1.0525x over previous
"""Trainium2 Bass kernel for Enformer-style relative-position attention.

Problem: b=2, n=1536, dim=1536, 8 heads, dk=64, dv=192, rel-pos features=192.

Sharding: (batch x head-pair), no collectives. 8 cores = 2 batches x 4
head-pairs. Each core computes q/k/v projections for its 2 heads only,
attention over all 1536 query rows, and a partial output projection
(contraction over its 384 feature columns of Wout). The host sums the 4
partial outputs per batch and adds b_out.

All matmuls run in bf16 (full PE rate). rel_k^T (128 x 3071) is computed
once on-device; per-tile pre-shift bands are column slices of it. The
shifted band is folded into the content-logit PSUM with an
identity-matmul accumulation (psum += I @ rbt), so exp reads PSUM
directly and no separate vector add pass exists. attnv lags one block
behind the logit pipeline so the PE never waits for exp/transpose.

relative_shift is realized as a DRAM round trip: the pre-shift band
S_pre (128 x 1663) is written row-major to a flat DRAM scratch, and read
back with row stride 1662 starting at offset 127, which lands
shifted[p, j] = S_pre[p, 127 - p + j] exactly.
"""

import math
import os

import ml_dtypes
import numpy as np

import concourse.bass as bass
import concourse.mybir as mybir
import concourse.tile as tile
from concourse import bacc
from concourse.bass_utils import run_bass_kernel_spmd
from concourse.masks import make_identity

BF16 = ml_dtypes.bfloat16
F32 = mybir.dt.float32
BF = mybir.dt.bfloat16
AF = mybir.ActivationFunctionType

P = 128
N = 1536          # sequence length
D = 1536          # model dim
H = 8             # heads
DK = 64           # key dim per head
DV = 192          # value dim per head
NT = N // P       # q-tiles per core = 12
KC = D // P       # contraction chunks over model dim = 12
WB = N + P - 1    # pre-shift band width = 1663
RBS = WB - 1      # readback row stride = 1662
FLAT = P * WB     # flat scratch elements per (tile, head) = 212864
NPOS = 2 * N - 1  # 3071 relative positions
NRF = 192         # rel-pos feature size
HPD = 2 * DV      # head-pair value width = 384
PRE = 6           # stg blocks pre-issued ahead of the consume loop
NBLK = 2 * NT     # (tile, head) blocks per core = 24


def _np_positions():
    """numpy replication of reference.get_positional_embed(1536, 192)."""
    from scipy.special import gammaln as sp_gammaln

    n, feature_size = N, NRF
    dist = np.arange(-n + 1, n)
    adist = np.abs(dist).astype(np.float64)[:, None]
    num_basis = feature_size // 6
    max_range = math.log(n) / math.log(2.0)
    half_life = 2.0 ** np.linspace(3.0, max_range, num_basis)
    feat_exp = np.exp(-math.log(2.0) / half_life[None, :] * adist)
    center_widths = 2.0 ** np.arange(1, num_basis + 1) - 1.0
    feat_cm = (center_widths[None, :] > adist).astype(np.float64)
    stddev = n / (2 * num_basis)
    start_mean = n / num_basis
    mean = np.linspace(start_mean, float(n), num_basis)[None, :]
    concentration = (mean / stddev) ** 2
    rate = mean / (stddev**2)
    with np.errstate(divide="ignore", invalid="ignore"):
        xl = np.where(
            adist > 0,
            (concentration - 1.0) * np.log(np.where(adist > 0, adist, 1.0)),
            0.0,
        )
        xl = np.where((adist == 0) & (concentration - 1.0 != 0), -np.inf, xl)
    log_unnorm = xl - rate * adist
    log_norm = sp_gammaln(concentration) - concentration * np.log(rate)
    prob = np.exp(log_unnorm - log_norm) + 1e-8
    feat_gamma = prob / np.max(prob, axis=-1, keepdims=True)
    emb = np.concatenate([feat_exp, feat_cm, feat_gamma], axis=-1)
    sign = np.sign(dist).astype(np.float64)[:, None]
    return np.concatenate([emb, sign * emb], axis=-1).astype(np.float32)


def _build_nc():
    nc = bacc.Bacc("TRN2", target_bir_lowering=False)

    xt = nc.dram_tensor("xt", [D, N], BF, kind="ExternalInput")
    wq = nc.dram_tensor("wq", [D, P], BF, kind="ExternalInput")
    wk = nc.dram_tensor("wk", [D, P], BF, kind="ExternalInput")
    wv = nc.dram_tensor("wv", [D, HPD], BF, kind="ExternalInput")
    wo = nc.dram_tensor("wo", [HPD, D], BF, kind="ExternalInput")
    wra = nc.dram_tensor("wra", [P, P], BF, kind="ExternalInput")
    wrb = nc.dram_tensor("wrb", [NRF - P, P], BF, kind="ExternalInput")
    pta = nc.dram_tensor("pta", [P, NPOS], BF, kind="ExternalInput")
    ptb = nc.dram_tensor("ptb", [NRF - P, NPOS], BF, kind="ExternalInput")
    rcb = nc.dram_tensor("rcb", [P, 1], F32, kind="ExternalInput")
    rpb = nc.dram_tensor("rpb", [P, 1], F32, kind="ExternalInput")

    out = nc.dram_tensor("out", [N, D], BF, kind="ExternalOutput")

    scale = DK ** -0.5

    with tile.TileContext(nc) as tc:
        with (
            tc.tile_pool(name="res", bufs=1) as res,
            tc.tile_pool(name="ps_wide", bufs=1, space="PSUM") as pwide,
            tc.tile_pool(name="ps_stg", bufs=1, space="PSUM") as pstgp,
            tc.tile_pool(name="ps_sm", bufs=2, space="PSUM") as psml,
            tc.tile_pool(name="dram", bufs=8, space="DRAM") as dsc,
            tc.tile_pool(name="stg", bufs=7) as stgp,
            tc.tile_pool(name="rb", bufs=8) as rbp,
            tc.tile_pool(name="es", bufs=3) as esp,
            tc.tile_pool(name="at", bufs=4) as atp,
            tc.tile_pool(name="av", bufs=2) as avp,
            tc.tile_pool(name="avT", bufs=2) as avTp,
            tc.tile_pool(name="ou", bufs=2) as oup,
            tc.tile_pool(name="sm", bufs=4) as smp,
        ):
            # long-lived SBUF residents
            kT_sb = res.tile([P, N], BF, tag="kT")            # 3 KB/part
            qcT_sb = res.tile([P, N], BF, tag="qcT")          # 3 KB/part
            qpT_sb = res.tile([P, N], BF, tag="qpT")          # 3 KB/part
            v_sb = res.tile([P, KC * HPD], BF, tag="v")       # 9 KB/part
            wo_sb = res.tile([P, 3 * D], BF, tag="wo")        # 9 KB/part
            relk_sb = res.tile([P, NPOS + 1], BF, tag="relk")  # 6 KB/part
            ident_sb = res.tile([P, P], BF, tag="ident")
            rcb_sb = res.tile([P, 1], F32, tag="rcb")
            rpb_sb = res.tile([P, 1], F32, tag="rpb")

            make_identity(nc, ident_sb[:])

            rbts = {}
            slots = {}
            stg_tiles = {}

            def emit_stg_round(i, r):
                # pre-shift rel logits S_pre (128, 1663) in two psum rounds
                t, hl = divmod(i, 2)
                ho = hl * DK
                lo = (N - 1) - t * P - (P - 1)
                if r == 0:
                    stg_tiles[i] = stgp.tile([P, WB + 1], BF, tag="stg", name=f"stg{i}")
                stg = stg_tiles[i]
                ps = pstgp.tile([P, 1024], F32, tag="pstg", name=f"pstg{i}_{r}")
                base = r * 1024
                for c in range(2):
                    w = min(512, WB - base - c * 512)
                    nc.tensor.matmul(
                        ps[:, c * 512 : c * 512 + w],
                        qpT_sb[ho : ho + DK, t * P : (t + 1) * P],
                        relk_sb[ho : ho + DK, lo + base + c * 512 : lo + base + c * 512 + w],
                        start=True, stop=True,
                    )
                wr = min(1024, WB - base)
                nc.vector.tensor_copy(stg[:, base : base + wr], ps[:, :wr])
                if r == 1:
                    slot = dsc.tile([FLAT], BF, tag="slot", name=f"slot{i}")
                    nc.gpsimd.dma_start(
                        out=slot[:].rearrange("(p w) -> p w", w=WB), in_=stg[:, :WB]
                    )
                    slots[i] = slot

            def emit_readback(i):
                rbt = rbp.tile([P, N], BF, tag="rb", name=f"rbt{i}")
                nc.sync.dma_start(
                    out=rbt[:],
                    in_=slots.pop(i)[P - 1 : P - 1 + P * RBS].rearrange(
                        "(p w) -> p w", w=RBS
                    )[:, :N],
                )
                rbts[i] = rbt

            with tc.tile_pool(name="proj", bufs=1) as projp:
                xT_sb = projp.tile([P, KC * N], BF, tag="xT")       # 36 KB/part
                wq_sb = projp.tile([P, KC * P], BF, tag="wq")
                wk_sb = projp.tile([P, KC * P], BF, tag="wk")
                wv_sb = projp.tile([P, KC * HPD], BF, tag="wv")
                pta_sb = projp.tile([P, NPOS], BF, tag="pta")
                ptb_sb = projp.tile([NRF - P, NPOS], BF, tag="ptb")
                wra_sb = projp.tile([P, P], BF, tag="wra")
                wrb_sb = projp.tile([NRF - P, P], BF, tag="wrb")

                # scalar queue: q/k weights first so projections start early
                nc.scalar.dma_start(out=rcb_sb[:], in_=rcb[:])
                nc.scalar.dma_start(out=rpb_sb[:], in_=rpb[:])
                nc.scalar.dma_start(
                    out=wq_sb[:].rearrange("p (kc c) -> p kc c", c=P),
                    in_=wq[:].rearrange("(kc p) c -> p kc c", p=P),
                )
                nc.scalar.dma_start(
                    out=wk_sb[:].rearrange("p (kc c) -> p kc c", c=P),
                    in_=wk[:].rearrange("(kc p) c -> p kc c", p=P),
                )
                nc.scalar.dma_start(out=wra_sb[:], in_=wra[:])
                nc.scalar.dma_start(out=wrb_sb[:], in_=wrb[:])
                nc.scalar.dma_start(out=pta_sb[:], in_=pta[:])
                nc.scalar.dma_start(out=ptb_sb[:], in_=ptb[:])
                # sync queue: x chunks, then late weights
                for kc in range(KC):
                    nc.sync.dma_start(
                        out=xT_sb[:, kc * N : (kc + 1) * N],
                        in_=xt[kc * P : (kc + 1) * P, :],
                    )
                nc.sync.dma_start(
                    out=wv_sb[:].rearrange("p (kc c) -> p kc c", c=HPD),
                    in_=wv[:].rearrange("(kc p) c -> p kc c", p=P),
                )
                nc.sync.dma_start(
                    out=wo_sb[:].rearrange("p (cc c) -> p cc c", c=D),
                    in_=wo[:].rearrange("(cc p) c -> p cc c", p=P),
                )

                # ---- q projection (2 heads, all rows): psum [128, 1536] ----
                pq = pwide.tile([P, 3 * 512], F32, tag="pwide", name="pq")
                for kc in range(KC):
                    for c3 in range(3):
                        nc.tensor.matmul(
                            pq[:, c3 * 512 : (c3 + 1) * 512],
                            wq_sb[:, kc * P : (kc + 1) * P],
                            xT_sb[:, kc * N + c3 * 512 : kc * N + (c3 + 1) * 512],
                            start=(kc == 0), stop=(kc == KC - 1),
                        )
                nc.scalar.activation(
                    qcT_sb[:], pq[:], AF.Identity, bias=rcb_sb[:, 0:1], scale=scale,
                )
                nc.scalar.activation(
                    qpT_sb[:], pq[:], AF.Identity, bias=rpb_sb[:, 0:1], scale=scale,
                )

                # ---- k projection ----
                for c3 in range(3):
                    pk = psml.tile([P, 512], F32, tag="psml", name=f"pk{c3}")
                    for kc in range(KC):
                        nc.tensor.matmul(
                            pk[:],
                            wk_sb[:, kc * P : (kc + 1) * P],
                            xT_sb[:, kc * N + c3 * 512 : kc * N + (c3 + 1) * 512],
                            start=(kc == 0), stop=(kc == KC - 1),
                        )
                    nc.vector.tensor_copy(kT_sb[:, c3 * 512 : (c3 + 1) * 512], pk[:])

                # ---- rel_k projection: relk = Wrel_slice @ positions^T ----
                for c in range(6):
                    co = c * 512
                    w = min(512, NPOS - co)
                    pr = psml.tile([P, 512], F32, tag="psml", name=f"pr{c}")
                    nc.tensor.matmul(
                        pr[:, :w], wra_sb[:], pta_sb[:, co : co + w],
                        start=True, stop=False,
                    )
                    nc.tensor.matmul(
                        pr[:, :w], wrb_sb[:], ptb_sb[:, co : co + w],
                        start=False, stop=True,
                    )
                    nc.scalar.copy(relk_sb[:, co : co + w], pr[:, :w])

                # ---- v projection woven with pre-issued stg rounds ----
                for m in range(KC):
                    pvm = psml.tile([P, 512], F32, tag="psml", name=f"pv{m}")
                    for kc in range(KC):
                        nc.tensor.matmul(
                            pvm[:, :HPD],
                            xT_sb[:, kc * N + m * P : kc * N + (m + 1) * P],
                            wv_sb[:, kc * HPD : (kc + 1) * HPD],
                            start=(kc == 0), stop=(kc == KC - 1),
                        )
                    nc.vector.tensor_copy(v_sb[:, m * HPD : (m + 1) * HPD], pvm[:, :HPD])
                    if m < 2 * PRE:
                        emit_stg_round(m // 2, m % 2)
                        if m % 2 == 1:
                            emit_readback(m // 2)

            # ------------- attention + output, pipelined blocks -------------
            # block i computes logits+exp for (t, hl) = divmod(i, 2); attnv
            # for block i-2 (the two-block lag covers the exp->transpose
            # chain); output projection for tile t' after its second head's
            # attnv completes.
            att_alls = {}
            recips = {}
            avs = {}
            LAG = 2

            for i in range(NBLK + LAG):
                if i < NBLK:
                    t, hl = divmod(i, 2)
                    ho = hl * DK
                    if i + PRE < NBLK:
                        emit_stg_round(i + PRE, 0)
                    # content logits into wide psum, band added via identity
                    pc = pwide.tile([P, 3 * 512], F32, tag="pwide", name=f"pc{i}")
                    for c3 in range(3):
                        nc.tensor.matmul(
                            pc[:, c3 * 512 : (c3 + 1) * 512],
                            qcT_sb[ho : ho + DK, t * P : (t + 1) * P],
                            kT_sb[ho : ho + DK, c3 * 512 : (c3 + 1) * 512],
                            start=True, stop=False,
                        )
                    rbt = rbts.pop(i)
                    for c3 in range(3):
                        sl = slice(c3 * 512, (c3 + 1) * 512)
                        nc.tensor.matmul(
                            pc[:, sl], ident_sb[:], rbt[:, sl],
                            start=False, stop=True,
                        )
                    if i + PRE < NBLK:
                        emit_stg_round(i + PRE, 1)
                    es = esp.tile([P, N], BF, tag="es", name=f"es{i}")
                    sums = smp.tile([P, 1], F32, tag="sums", name=f"sums{i}")
                    nc.scalar.activation(es[:], pc[:], AF.Exp, accum_out=sums[:])
                    recip = smp.tile([P, 1], F32, tag="recip", name=f"recip{i}")
                    nc.vector.reciprocal(recip[:], sums[:])
                    recips[i] = recip
                    # transpose in two halves so attnv can start on the
                    # first 6 key-chunks early; readback for block i+PRE
                    # queues on sync behind them (it has PRE blocks slack)
                    att_all = atp.tile([P, KC * P], BF, tag="at", name=f"at{i}")
                    nc.sync.dma_start_transpose(
                        att_all[:, : 6 * P].rearrange("p (kc c) -> p kc c", c=P),
                        es[:, : 6 * P],
                    )
                    nc.sync.dma_start_transpose(
                        att_all[:, 6 * P :].rearrange("p (kc c) -> p kc c", c=P),
                        es[:, 6 * P :],
                    )
                    att_alls[i] = att_all
                    if i + PRE < NBLK:
                        emit_readback(i + PRE)

                if i >= LAG:
                    j = i - LAG
                    tj, hj = divmod(j, 2)
                    if hj == 0:
                        avs[tj] = avp.tile([P, HPD], BF, tag="attnv", name=f"av{tj}")
                    att_all = att_alls.pop(j)
                    pv = psml.tile([P, 512], F32, tag="psml", name=f"pav{j}")
                    for kc in range(KC):
                        nc.tensor.matmul(
                            pv[:, :DV],
                            att_all[:, kc * P : (kc + 1) * P],
                            v_sb[:, kc * HPD + hj * DV : kc * HPD + (hj + 1) * DV],
                            start=(kc == 0), stop=(kc == KC - 1),
                        )
                    nc.scalar.activation(
                        avs[tj][:, hj * DV : (hj + 1) * DV], pv[:, :DV], AF.Copy,
                        scale=recips.pop(j)[:],
                    )
                    if hj == 1:
                        # ---- partial output projection for tile tj ----
                        attnv_sb = avs.pop(tj)
                        avT = avTp.tile([P, 3 * P], BF, tag="avT", name=f"avT{tj}")
                        nc.sync.dma_start_transpose(
                            avT[:].rearrange("p (cc c) -> p cc c", c=P), attnv_sb[:]
                        )
                        ot = oup.tile([P, D], BF, tag="ot", name=f"ot{tj}")
                        for c3 in range(3):
                            po = psml.tile([P, 512], F32, tag="psml", name=f"po{tj}_{c3}")
                            for cc in range(3):
                                nc.tensor.matmul(
                                    po[:],
                                    avT[:, cc * P : (cc + 1) * P],
                                    wo_sb[:, cc * D + c3 * 512 : cc * D + (c3 + 1) * 512],
                                    start=(cc == 0), stop=(cc == 2),
                                )
                            nc.scalar.activation(
                                ot[:, c3 * 512 : (c3 + 1) * 512], po[:], AF.Copy,
                            )
                        nc.gpsimd.dma_start(
                            out=out[tj * P : (tj + 1) * P, :], in_=ot[:]
                        )

    nc.compile()
    return nc


_CACHE = {}


def _get_nc():
    if "nc" not in _CACHE:
        _CACHE["nc"] = _build_nc()
    return _CACHE["nc"]


def kernel(x, Wq, Wk, Wv, Wrel, Wout, b_out, rel_content_bias, rel_pos_bias):
    x = np.asarray(x, np.float32)
    Wq = np.asarray(Wq, np.float32)
    Wk = np.asarray(Wk, np.float32)
    Wv = np.asarray(Wv, np.float32)
    Wrel = np.asarray(Wrel, np.float32)
    Wout = np.asarray(Wout, np.float32)
    b_out = np.asarray(b_out, np.float32)
    rcb = np.asarray(rel_content_bias, np.float32).reshape(H, DK)
    rpb = np.asarray(rel_pos_bias, np.float32).reshape(H, DK)

    positions = _np_positions()  # (3071, 192) f32, input-independent constant
    posT = np.ascontiguousarray(positions.T).astype(BF16)  # (192, 3071)

    in_maps = []
    for core in range(8):
        bi, hp = core // 4, core % 4
        h0 = 2 * hp
        xT = np.ascontiguousarray(x[bi].T).astype(BF16)  # (dim, n)
        wrT = np.ascontiguousarray(
            Wrel[h0 * DK : (h0 + 2) * DK, :].T
        ).astype(BF16)  # (192, 128)
        in_maps.append(
            {
                "xt": xT,
                "wq": np.ascontiguousarray(Wq[h0 * DK : (h0 + 2) * DK, :].T).astype(BF16),
                "wk": np.ascontiguousarray(Wk[h0 * DK : (h0 + 2) * DK, :].T).astype(BF16),
                "wv": np.ascontiguousarray(Wv[h0 * DV : (h0 + 2) * DV, :].T).astype(BF16),
                "wo": np.ascontiguousarray(Wout[:, h0 * DV : (h0 + 2) * DV].T).astype(BF16),
                "wra": np.ascontiguousarray(wrT[:P]),
                "wrb": np.ascontiguousarray(wrT[P:]),
                "pta": np.ascontiguousarray(posT[:P]),
                "ptb": np.ascontiguousarray(posT[P:]),
                "rcb": np.ascontiguousarray(rcb[h0 : h0 + 2].reshape(P, 1)),
                "rpb": np.ascontiguousarray(rpb[h0 : h0 + 2].reshape(P, 1)),
            }
        )

    nc = _get_nc()
    trace = bool(os.environ.get("KERNEL_TRACE"))
    res = run_bass_kernel_spmd(nc, in_maps, list(range(8)), trace=trace)
    _CACHE["last_res"] = res

    out = np.zeros((2, N, D), np.float32)
    for core in range(8):
        bi = core // 4
        out[bi] += np.asarray(res.results[core]["out"]).astype(np.float32)
    out += b_out
    return out
